# revision 8
# baseline (speedup 1.0000x reference)
"""MHSA on 8 TRN2 NeuronCores.

Sharding: core c handles batch b=c//4, query rows [qi*1024,(qi+1)*1024) with
qi=c%4, all 8 heads. K/V are recomputed per core over the full sequence of
its batch (duplicated across the 4 cores of a batch) -> zero collectives.
Host rotates x so each core's query slice is always columns 0:1024.

v2 schedule: ACT (exp) is the roofline engine (~267us). Projections are
interleaved INTO the attention phases so PE never runs a serial prologue:
  upfront: K^T/Q^T m=0 chunks only (pa-tag psum ring)
  phase 0: scores/exp + one V tile per group
  phase 1: + K^T/Q^T m=1 chunks   (needed at phase 2)
  phases 2-3: + m=2 chunks        (needed at phase 4)
  phases 4-5: + m=3 chunks        (needed at phase 6)
Scores run as two concurrent 64x128 PE row-tiles (auto tile_position);
groups are emitted in stints of 2 (SS..AAAA) to halve mode-switch drains.
attn^T accumulates one phase behind; V carries a ones column so row 64 of
the psum is the softmax denominator Z. zinv is broadcast across 64
partitions via a bf16 K=1 matmul. Out = attnT.T @ Wo -> dram fp32.
"""

import numpy as np

B, S, D, H, DH = 2, 4096, 512, 8, 64
SQ = S // 4          # q rows per core
NKT = S // 128       # 32 k-tiles
NDC = D // 128       # 4 contraction chunks
PHASES = [(pair, qt) for pair in range(4) for qt in range(2)]

_CACHE = {}


def _build_nc():
    import concourse.bacc as bacc
    import concourse.bass as bass
    from concourse import mybir, tile

    F32 = mybir.dt.float32
    BF = mybir.dt.bfloat16
    EXP = mybir.ActivationFunctionType.Exp

    nc = bacc.Bacc(target_bir_lowering=False)
    xt_d = nc.declare_dram_parameter("xt", [D, S], BF, isOutput=False)
    wq_d = nc.declare_dram_parameter("wq", [D, D], BF, isOutput=False)
    wk_d = nc.declare_dram_parameter("wk", [D, D], BF, isOutput=False)
    wv_d = nc.declare_dram_parameter("wv", [D, D], BF, isOutput=False)
    wo_d = nc.declare_dram_parameter("wo", [D, D], BF, isOutput=False)
    out_d = nc.declare_dram_parameter("out", [SQ, D], F32, isOutput=True)

    with tile.TileContext(nc) as tc:
        with (
            tc.tile_pool(name="const", bufs=1) as cpool,
            tc.tile_pool(name="kv", bufs=1) as kvpool,
            tc.tile_pool(name="p", bufs=34) as ppool,
            tc.tile_pool(name="z", bufs=1) as zpool,
            tc.tile_pool(name="osb", bufs=2) as opool,
            tc.tile_pool(name="ps_s", bufs=2, space=bass.MemorySpace.PSUM) as sp,
            tc.tile_pool(name="ps_m", bufs=2, space=bass.MemorySpace.PSUM) as mp,
            tc.tile_pool(name="ps_p", bufs=1, space=bass.MemorySpace.PSUM) as pp,
        ):
            # ---- loads (wk + xt first: needed by the upfront m=0 chunks) ----
            w_sb = {}

            def load_w(nm, dram):
                w_sb[nm] = []
                for dc in range(NDC):
                    t = cpool.tile([128, D], BF, name=f"w_{nm}_{dc}")
                    nc.sync.dma_start(t[:], dram[dc * 128:(dc + 1) * 128, :])
                    w_sb[nm].append(t)

            load_w("wk", wk_d)
            xt_sb = []
            for dc in range(NDC):
                t = cpool.tile([128, S], BF, name=f"xt_{dc}")
                # split loads so the first projection chunks only wait on the
                # first quarter of each xt row-block
                for cc in range(4):
                    c0, c1 = cc * (S // 4), (cc + 1) * (S // 4)
                    nc.sync.dma_start(t[:, c0:c1],
                                      xt_d[dc * 128:(dc + 1) * 128, c0:c1])
                xt_sb.append(t)
            load_w("wq", wq_d)
            load_w("wv", wv_d)
            load_w("wo", wo_d)

            ones64 = cpool.tile([1, 64], BF, name="ones64")
            nc.gpsimd.memset(ones64[:], 1.0)
            zbias = cpool.tile([128, 1], F32, name="zbias")
            nc.gpsimd.memset(zbias[:], 0.0)

            kT_sb = [kvpool.tile([128, S], BF, name=f"kT_{m}") for m in range(NDC)]
            qT_sb = [kvpool.tile([128, SQ], BF, name=f"qT_{m}") for m in range(NDC)]
            v_sb = [kvpool.tile([128, H, DH + 1], BF, name=f"v_{st}")
                    for st in range(NKT)]
            attnT_sb = [kvpool.tile([128, SQ], BF, name=f"attnT_{m}")
                        for m in range(NDC)]

            # ---- projection chunk emitters ----
            # early (pre-phase-1) chunks share the pa-tag psum ring (free until
            # attn(0) first writes it in phase 1); later chunks get their own
            # 1-deep bank so they never touch the live attn accumulators.
            def proj_ps(early):
                if early:
                    return mp.tile([128, 512], F32, name="pa")
                return pp.tile([128, 512], F32, name="pp")

            def emit_kqt(wname, dst, m, st, early):
                ps = proj_ps(early)
                for dc in range(NDC):
                    nc.tensor.matmul(
                        ps[:],
                        w_sb[wname][dc][:, m * 128:(m + 1) * 128],
                        xt_sb[dc][:, st * 512:(st + 1) * 512],
                        start=(dc == 0), stop=(dc == NDC - 1),
                    )
                nc.vector.tensor_copy(dst[:, st * 512:(st + 1) * 512], ps[:])

            def emit_v(st, early):
                vt = v_sb[st]
                nc.gpsimd.memset(vt[:, :, DH:DH + 1], 1.0)
                ps = proj_ps(early)
                for dc in range(NDC):
                    nc.tensor.matmul(
                        ps[:],
                        xt_sb[dc][:, st * 128:(st + 1) * 128],
                        w_sb["wv"][dc][:],
                        start=(dc == 0), stop=(dc == NDC - 1),
                    )
                nc.vector.tensor_copy(
                    vt[:, :, 0:DH], ps[:].rearrange("p (h d) -> p h d", h=H)
                )

            for st in range(S // 512):
                emit_kqt("wk", kT_sb[0], 0, st, early=True)
            for st in range(SQ // 512):
                emit_kqt("wq", qT_sb[0], 0, st, early=True)

            def kqt_tasks(m):
                return ([("k", m, st) for st in range(S // 512)]
                        + [("q", m, st) for st in range(SQ // 512)])

            sched = {p: [] for p in range(len(PHASES))}
            sched[0] = [("v", 0, st) for st in range(NKT)]
            sched[1] = kqt_tasks(1)
            t2, t3 = kqt_tasks(2), kqt_tasks(3)
            sched[2], sched[3] = t2[:5], t2[5:]
            sched[4], sched[5] = t3[:5], t3[5:]

            def emit_task(task, early):
                kind, m, st = task
                if kind == "v":
                    emit_v(st, early)
                elif kind == "k":
                    emit_kqt("wk", kT_sb[m], m, st, early)
                else:
                    emit_kqt("wq", qT_sb[m], m, st, early)

            # ---- attention ----
            def emit_attn(prev_pa, prev_pt, pp_idx, g):
                pair_, _qt = PHASES[pp_idx]
                for h01 in range(2):
                    h = 2 * pair_ + h01
                    nc.tensor.matmul(
                        prev_pa[h01][0:DH + 1, :],
                        v_sb[g][:, h, :],
                        prev_pt[g][:, h01 * 512:(h01 + 1) * 512],
                        start=(g == 0), stop=(g == NKT - 1),
                    )

            # Epilogue is split: the two slow 1-partition DVE reciprocals are
            # emitted at the START of the following phase (they only need the
            # finished pa accumulators, and DVE is idle then), while the PE
            # broadcast + mul land EPI stints later, by which time the recips
            # have completed -> no PE stall waiting on DVE.
            def emit_epi_recips(pa2):
                zinvs = []
                for h01 in range(2):
                    zinv = zpool.tile([1, 512], BF, name="zinv", bufs=2)
                    with nc.allow_low_precision(reason="bf16 zinv broadcast"):
                        nc.vector.reciprocal(zinv[:], pa2[h01][DH:DH + 1, :])
                    zinvs.append(zinv)
                return zinvs

            def emit_epi_rest(zinvs, pa2, pp_idx):
                pair_, qt_ = PHASES[pp_idx]
                for h01 in range(2):
                    zb_ps = mp.tile([128, 512], F32, name="zb_ps", bufs=1)
                    nc.tensor.matmul(
                        zb_ps[0:64, :], ones64[0:1, :], zinvs[h01][0:1, :],
                        start=True, stop=True,
                    )
                    zb_sb = zpool.tile([64, 512], F32, name="zb_sb", bufs=2)
                    nc.vector.tensor_copy(zb_sb[:], zb_ps[0:64, :])
                    nc.vector.tensor_mul(
                        attnT_sb[pair_][h01 * 64:(h01 + 1) * 64,
                                        qt_ * 512:(qt_ + 1) * 512],
                        pa2[h01][0:64, :],
                        zb_sb[:],
                    )

            # out-proj reuses the (retired) scores psum ring; copies go on the
            # scalar engine so the tail DVE reciprocals don't block them
            def emit_outproj(qb):
                ps = sp.tile([128, 1024], F32, name="ps_scores")
                for m in range(NDC):
                    nc.tensor.matmul(
                        ps[:, 0:512],
                        attnT_sb[m][:, qb * 128:(qb + 1) * 128],
                        w_sb["wo"][m][:],
                        start=(m == 0), stop=(m == NDC - 1),
                    )
                ot = opool.tile([128, 512], F32, name="ot")
                nc.scalar.copy(ot[:], ps[:, 0:512])
                nc.sync.dma_start(out_d[qb * 128:(qb + 1) * 128, :], ot[:])

            EPI = 8
            prev_pa = prev_pt = pa2 = zinvs = None
            for p, (pair, qt) in enumerate(PHASES):
                tasks = sched[p]
                spacing = max(1, NKT // max(1, len(tasks)))
                task_map = {i * spacing: t for i, t in enumerate(tasks)}
                pt_list = []
                for g2 in range(0, NKT, 2):
                    if p >= 2 and g2 == 0:
                        zinvs = emit_epi_recips(pa2)
                    for g in (g2, g2 + 1):
                        ps = sp.tile([128, 1024], F32, name="ps_scores")
                        for h01 in range(2):
                            nc.tensor.matmul(
                                ps[:, h01 * 512:(h01 + 1) * 512],
                                kT_sb[pair][h01 * 64:(h01 + 1) * 64,
                                            g * 128:(g + 1) * 128],
                                qT_sb[pair][h01 * 64:(h01 + 1) * 64,
                                            qt * 512:(qt + 1) * 512],
                                start=True, stop=True,
                            )
                        pt = ppool.tile([128, 1024], BF, name="ptile")
                        nc.scalar.activation(pt[:], ps[:], EXP, bias=zbias[:],
                                             scale=0.125)
                        pt_list.append(pt)
                    if p > 0:
                        emit_attn(prev_pa, prev_pt, p - 1, g2)
                        emit_attn(prev_pa, prev_pt, p - 1, g2 + 1)
                    if p >= 2 and g2 == EPI:
                        emit_epi_rest(zinvs, pa2, p - 2)
                    for g in (g2, g2 + 1):
                        if g in task_map:
                            emit_task(task_map[g], early=(p == 0))
                pa = [mp.tile([128, 512], F32, name="pa") for i in range(2)]
                pa2, prev_pa, prev_pt = prev_pa, pa, pt_list

            # ---- drain: attn for the last phase + deferred epilogue(6) ----
            for g2 in range(0, NKT, 2):
                if g2 == 0:
                    zinvs = emit_epi_recips(pa2)
                emit_attn(prev_pa, prev_pt, len(PHASES) - 1, g2)
                emit_attn(prev_pa, prev_pt, len(PHASES) - 1, g2 + 1)
                if g2 == 16:
                    emit_epi_rest(zinvs, pa2, len(PHASES) - 2)

            # ---- tail: epilogue(7) recips overlap out-proj of the qt=0 half
            zinvs = emit_epi_recips(prev_pa)
            for qb in range(4):
                emit_outproj(qb)
            emit_epi_rest(zinvs, prev_pa, len(PHASES) - 1)
            for qb in range(4, SQ // 128):
                emit_outproj(qb)

    nc.compile()
    return nc


def kernel(inputs, Wq, bq, Wk, bk, Wv, bv, Wo, bo):
    import ml_dtypes
    from concourse.bass_utils import run_bass_kernel_spmd

    bf16 = ml_dtypes.bfloat16
    if "nc" not in _CACHE:
        _CACHE["nc"] = _build_nc()
    nc = _CACHE["nc"]

    x = np.asarray(inputs, dtype=np.float32)
    xT = [np.ascontiguousarray(x[b].T).astype(bf16) for b in range(B)]
    w = {k: np.ascontiguousarray(np.asarray(v, np.float32)).astype(bf16)
         for k, v in (("wq", Wq), ("wk", Wk), ("wv", Wv), ("wo", Wo))}

    in_maps = []
    for c in range(8):
        b, qi = c // 4, c % 4
        in_maps.append({
            "xt": np.ascontiguousarray(np.roll(xT[b], -qi * SQ, axis=1)),
            **w,
        })

    res = run_bass_kernel_spmd(nc, in_maps, list(range(8)))
    out = np.empty((B, S, D), np.float32)
    for c in range(8):
        b, qi = c // 4, c % 4
        out[b, qi * SQ:(qi + 1) * SQ, :] = res.results[c]["out"]
    return out


# revision 12
# speedup vs baseline: 1.1892x; 1.1892x over previous
"""MHSA on 8 TRN2 NeuronCores.

Sharding: core c handles batch b=c//4, query rows [qi*1024,(qi+1)*1024) with
qi=c%4, all 8 heads. K/V are recomputed per core over the full sequence of
its batch (duplicated across the 4 cores of a batch) -> zero collectives.
Host rotates x so each core's query slice is always columns 0:1024.

v2 schedule: ACT (exp) is the roofline engine (~267us). Projections are
interleaved INTO the attention phases so PE never runs a serial prologue:
  upfront: K^T/Q^T m=0 chunks only (pa-tag psum ring)
  phase 0: scores/exp + one V tile per group
  phase 1: + K^T/Q^T m=1 chunks   (needed at phase 2)
  phases 2-3: + m=2 chunks        (needed at phase 4)
  phases 4-5: + m=3 chunks        (needed at phase 6)
Scores run as two concurrent 64x128 PE row-tiles (auto tile_position);
groups are emitted in stints of 2 (SS..AAAA) to halve mode-switch drains.
attn^T accumulates one phase behind; V carries a ones column so row 64 of
the psum is the softmax denominator Z. zinv is broadcast across 64
partitions via a bf16 K=1 matmul. Out = attnT.T @ Wo -> dram fp32.
"""

import numpy as np

B, S, D, H, DH = 2, 4096, 512, 8, 64
SQ = S // 4          # q rows per core
NKT = S // 128       # 32 k-tiles
NDC = D // 128       # 4 contraction chunks
PHASES = [(pair, qt) for pair in range(4) for qt in range(2)]

_CACHE = {}


def _build_nc():
    import concourse.bacc as bacc
    import concourse.bass as bass
    from concourse import mybir, tile

    F32 = mybir.dt.float32
    BF = mybir.dt.bfloat16
    EXP = mybir.ActivationFunctionType.Exp

    nc = bacc.Bacc(target_bir_lowering=False)
    xt_d = nc.declare_dram_parameter("xt", [D, S], BF, isOutput=False)
    wq_d = nc.declare_dram_parameter("wq", [D, D], BF, isOutput=False)
    wk_d = nc.declare_dram_parameter("wk", [D, D], BF, isOutput=False)
    wv_d = nc.declare_dram_parameter("wv", [D, D], BF, isOutput=False)
    wo_d = nc.declare_dram_parameter("wo", [D, D], BF, isOutput=False)
    out_d = nc.declare_dram_parameter("out", [SQ, D], F32, isOutput=True)

    with tile.TileContext(nc) as tc:
        with (
            tc.tile_pool(name="const", bufs=1) as cpool,
            tc.tile_pool(name="kv", bufs=1) as kvpool,
            tc.tile_pool(name="p", bufs=34) as ppool,
            tc.tile_pool(name="z", bufs=1) as zpool,
            tc.tile_pool(name="osb", bufs=2) as opool,
            tc.tile_pool(name="ps_s", bufs=2, space=bass.MemorySpace.PSUM) as sp,
            tc.tile_pool(name="ps_m", bufs=2, space=bass.MemorySpace.PSUM) as mp,
            tc.tile_pool(name="ps_p", bufs=1, space=bass.MemorySpace.PSUM) as pp,
        ):
            # ---- loads (wk + xt first: needed by the upfront m=0 chunks) ----
            w_sb = {}

            def load_w(nm, dram):
                w_sb[nm] = []
                for dc in range(NDC):
                    t = cpool.tile([128, D], BF, name=f"w_{nm}_{dc}")
                    nc.sync.dma_start(t[:], dram[dc * 128:(dc + 1) * 128, :])
                    w_sb[nm].append(t)

            load_w("wk", wk_d)
            xt_sb = []
            for dc in range(NDC):
                t = cpool.tile([128, S], BF, name=f"xt_{dc}")
                # split loads so the first projection chunks only wait on the
                # first quarter of each xt row-block
                for cc in range(4):
                    c0, c1 = cc * (S // 4), (cc + 1) * (S // 4)
                    nc.sync.dma_start(t[:, c0:c1],
                                      xt_d[dc * 128:(dc + 1) * 128, c0:c1])
                xt_sb.append(t)
            load_w("wq", wq_d)
            load_w("wv", wv_d)
            load_w("wo", wo_d)

            ones64 = cpool.tile([1, 64], BF, name="ones64")
            nc.gpsimd.memset(ones64[:], 1.0)
            zbias = cpool.tile([128, 1], F32, name="zbias")
            nc.gpsimd.memset(zbias[:], 0.0)

            kT_sb = [kvpool.tile([128, S], BF, name=f"kT_{m}") for m in range(NDC)]
            qT_sb = [kvpool.tile([128, SQ], BF, name=f"qT_{m}") for m in range(NDC)]
            v_sb = [kvpool.tile([128, H, DH + 1], BF, name=f"v_{st}")
                    for st in range(NKT)]
            attnT_sb = [kvpool.tile([128, SQ], BF, name=f"attnT_{m}")
                        for m in range(NDC)]

            # ---- projection chunk emitters ----
            # early (pre-phase-1) chunks share the pa-tag psum ring (free until
            # attn(0) first writes it in phase 1); later chunks get their own
            # 1-deep bank so they never touch the live attn accumulators.
            def proj_ps(early):
                if early:
                    return mp.tile([128, 512], F32, name="pa")
                return pp.tile([128, 512], F32, name="pp")

            def emit_kqt(wname, dst, m, st, early):
                ps = proj_ps(early)
                for dc in range(NDC):
                    nc.tensor.matmul(
                        ps[:],
                        w_sb[wname][dc][:, m * 128:(m + 1) * 128],
                        xt_sb[dc][:, st * 512:(st + 1) * 512],
                        start=(dc == 0), stop=(dc == NDC - 1),
                    )
                nc.vector.tensor_copy(dst[:, st * 512:(st + 1) * 512], ps[:])

            def emit_v(st, early):
                vt = v_sb[st]
                nc.gpsimd.memset(vt[:, :, DH:DH + 1], 1.0)
                ps = proj_ps(early)
                for dc in range(NDC):
                    nc.tensor.matmul(
                        ps[:],
                        xt_sb[dc][:, st * 128:(st + 1) * 128],
                        w_sb["wv"][dc][:],
                        start=(dc == 0), stop=(dc == NDC - 1),
                    )
                nc.vector.tensor_copy(
                    vt[:, :, 0:DH], ps[:].rearrange("p (h d) -> p h d", h=H)
                )

            for st in range(S // 512):
                emit_kqt("wk", kT_sb[0], 0, st, early=True)
            for st in range(SQ // 512):
                emit_kqt("wq", qT_sb[0], 0, st, early=True)

            def kqt_tasks(m):
                return ([("k", m, st) for st in range(S // 512)]
                        + [("q", m, st) for st in range(SQ // 512)])

            sched = {p: [] for p in range(len(PHASES))}
            sched[0] = [("v", 0, st) for st in range(NKT)]
            sched[1] = kqt_tasks(1)
            t2, t3 = kqt_tasks(2), kqt_tasks(3)
            sched[2], sched[3] = t2[:5], t2[5:]
            sched[4], sched[5] = t3[:5], t3[5:]

            def emit_task(task, early):
                kind, m, st = task
                if kind == "v":
                    emit_v(st, early)
                elif kind == "k":
                    emit_kqt("wk", kT_sb[m], m, st, early)
                else:
                    emit_kqt("wq", qT_sb[m], m, st, early)

            # ---- attention ----
            def emit_attn(prev_pa, prev_pt, pp_idx, g):
                pair_, _qt = PHASES[pp_idx]
                for h01 in range(2):
                    h = 2 * pair_ + h01
                    nc.tensor.matmul(
                        prev_pa[h01][0:DH + 1, :],
                        v_sb[g][:, h, :],
                        prev_pt[g][:, h01 * 512:(h01 + 1) * 512],
                        start=(g == 0), stop=(g == NKT - 1),
                    )

            # Epilogue is split: the two slow 1-partition DVE reciprocals are
            # emitted at the START of the following phase (they only need the
            # finished pa accumulators, and DVE is idle then), while the PE
            # broadcast + mul land EPI stints later, by which time the recips
            # have completed -> no PE stall waiting on DVE.
            def emit_epi_recips(pa2):
                zinvs = []
                for h01 in range(2):
                    zinv = zpool.tile([1, 512], BF, name="zinv", bufs=2)
                    with nc.allow_low_precision(reason="bf16 zinv broadcast"):
                        nc.vector.reciprocal(zinv[:], pa2[h01][DH:DH + 1, :])
                    zinvs.append(zinv)
                return zinvs

            def emit_epi_rest(zinvs, pa2, pp_idx):
                pair_, qt_ = PHASES[pp_idx]
                for h01 in range(2):
                    zb_ps = mp.tile([128, 512], F32, name="zb_ps", bufs=1)
                    nc.tensor.matmul(
                        zb_ps[0:64, :], ones64[0:1, :], zinvs[h01][0:1, :],
                        start=True, stop=True,
                    )
                    zb_sb = zpool.tile([64, 512], BF, name="zb_sb", bufs=2)
                    nc.vector.tensor_copy(zb_sb[:], zb_ps[0:64, :])
                    nc.vector.tensor_mul(
                        attnT_sb[pair_][h01 * 64:(h01 + 1) * 64,
                                        qt_ * 512:(qt_ + 1) * 512],
                        pa2[h01][0:64, :],
                        zb_sb[:],
                    )

            # out-proj reuses the (retired) scores psum ring; copies go on the
            # scalar engine so the tail DVE reciprocals don't block them
            def emit_outproj(qb):
                ps = sp.tile([128, 1024], F32, name="ps_scores")
                for m in range(NDC):
                    nc.tensor.matmul(
                        ps[:, 0:512],
                        attnT_sb[m][:, qb * 128:(qb + 1) * 128],
                        w_sb["wo"][m][:],
                        start=(m == 0), stop=(m == NDC - 1),
                    )
                ot = opool.tile([128, 512], F32, name="ot")
                nc.scalar.copy(ot[:], ps[:, 0:512])
                nc.sync.dma_start(out_d[qb * 128:(qb + 1) * 128, :], ot[:])

            EPI = 8
            prev_pa = prev_pt = pa2 = zinvs = None
            for p, (pair, qt) in enumerate(PHASES):
                tasks = sched[p]
                spacing = max(1, NKT // max(1, len(tasks)))
                task_map = {i * spacing: t for i, t in enumerate(tasks)}
                pt_list = []
                for g2 in range(0, NKT, 2):
                    if p >= 2 and g2 == 0:
                        zinvs = emit_epi_recips(pa2)
                    for g in (g2, g2 + 1):
                        ps = sp.tile([128, 1024], F32, name="ps_scores")
                        for h01 in range(2):
                            nc.tensor.matmul(
                                ps[:, h01 * 512:(h01 + 1) * 512],
                                kT_sb[pair][h01 * 64:(h01 + 1) * 64,
                                            g * 128:(g + 1) * 128],
                                qT_sb[pair][h01 * 64:(h01 + 1) * 64,
                                            qt * 512:(qt + 1) * 512],
                                start=True, stop=True,
                            )
                        pt = ppool.tile([128, 1024], BF, name="ptile")
                        nc.scalar.activation(pt[:], ps[:], EXP, bias=zbias[:],
                                             scale=0.125)
                        pt_list.append(pt)
                    if p > 0:
                        emit_attn(prev_pa, prev_pt, p - 1, g2)
                        emit_attn(prev_pa, prev_pt, p - 1, g2 + 1)
                    if p >= 2 and g2 == EPI:
                        emit_epi_rest(zinvs, pa2, p - 2)
                    for g in (g2, g2 + 1):
                        if g in task_map:
                            emit_task(task_map[g], early=(p == 0))
                pa = [mp.tile([128, 512], F32, name="pa") for i in range(2)]
                pa2, prev_pa, prev_pt = prev_pa, pa, pt_list

            # ---- drain: attn for the last phase + deferred epilogue(6) ----
            for g2 in range(0, NKT, 2):
                if g2 == 0:
                    zinvs = emit_epi_recips(pa2)
                emit_attn(prev_pa, prev_pt, len(PHASES) - 1, g2)
                emit_attn(prev_pa, prev_pt, len(PHASES) - 1, g2 + 1)
                if g2 == 16:
                    emit_epi_rest(zinvs, pa2, len(PHASES) - 2)

            # ---- tail: epilogue(7) recips overlap out-proj of the qt=0 half
            zinvs = emit_epi_recips(prev_pa)
            for qb in range(4):
                emit_outproj(qb)
            emit_epi_rest(zinvs, prev_pa, len(PHASES) - 1)
            for qb in range(4, SQ // 128):
                emit_outproj(qb)

    nc.compile()
    return nc


def kernel(inputs, Wq, bq, Wk, bk, Wv, bv, Wo, bo):
    import ml_dtypes
    from concourse.bass_utils import run_bass_kernel_spmd

    bf16 = ml_dtypes.bfloat16
    if "nc" not in _CACHE:
        _CACHE["nc"] = _build_nc()
    nc = _CACHE["nc"]

    x = np.asarray(inputs, dtype=np.float32)
    xT = [np.ascontiguousarray(x[b].T).astype(bf16) for b in range(B)]
    w = {k: np.ascontiguousarray(np.asarray(v, np.float32)).astype(bf16)
         for k, v in (("wq", Wq), ("wk", Wk), ("wv", Wv), ("wo", Wo))}

    in_maps = []
    for c in range(8):
        b, qi = c // 4, c % 4
        in_maps.append({
            "xt": np.ascontiguousarray(np.roll(xT[b], -qi * SQ, axis=1)),
            **w,
        })

    res = run_bass_kernel_spmd(nc, in_maps, list(range(8)))
    out = np.empty((B, S, D), np.float32)
    for c in range(8):
        b, qi = c // 4, c % 4
        out[b, qi * SQ:(qi + 1) * SQ, :] = res.results[c]["out"]
    return out
